# revision 18
# baseline (speedup 1.0000x reference)
"""Trainium2 Bass kernel for the per-head channel-attention module.

Math (per batch b, all fp32):
  Q = emb @ Wq[h].T, K = emb @ Wk[h].T        [N, C] each
  scores_h = Q.T @ K / sqrt(C)                [C, C]
  probs = softmax(InstanceNorm(scores), -1)
  weights = mean_h probs                      [C, C]   (output 2)
  O1 = (1/H sum_h probs_h @ V_h-chain) @ Wout [N, C]   (output 1)

Key restructure: scores contract over tokens N, so
  scores_h = Wq_h @ G' @ Wk_h.T  with  G' = (emb.T @ emb) / sqrt(C)
and the output path collapses to
  O1 = emb @ Z;  S'[i,c] = sum_h (probs_h @ Wv_h)[i,c];
  Z[c,d] = (1/H) sum_i S'[i,c] * Wout[d,i]
~11.3 GFLOP/batch -> ~1.5 GFLOP/batch.  InstanceNorm's mean subtraction
cancels inside the row softmax, so only r = rsqrt(var+eps) is needed.
G is exactly symmetric, so its lower-left block is a transpose copy.

Sharding: data-parallel, one batch per NeuronCore (B=8, 8 cores).
Host pre-transposes emb (embT) and pre-swizzles the weight matrices into
one SBUF-layout buffer so every DMA is a few large contiguous runs.
Per-core outputs: o1T = O1[b].T (host transposes back) and wts = weights[b].
"""

import os

import numpy as np

import concourse.bacc as bacc
import concourse.bass as bass
import concourse.mybir as mybir
import concourse.tile as tile
from concourse.bass import _add_dep_helper
from concourse.bass_utils import run_bass_kernel_spmd
from concourse.masks import make_identity

B, N, C, H = 8, 4096, 256, 4
EPS = 1e-5
P = 128
TC = C // P          # 2 c-tiles
KT = N // P          # 32 token-tiles
NCH = N // 512       # 8 chunks of 512 tokens for the final matmul
EMB_CHUNKS = 4
F32 = mybir.dt.float32

# Matmul dtype knobs. float32 = exact (4 cyc/row); float32r = fast (~2 cyc/row
# measured) with relaxed multiply precision (~2e-4 rel err end to end).
_DT_MAP = {"float32": mybir.dt.float32, "float32r": mybir.dt.float32r}
MM_BIG = _DT_MAP[os.environ.get("ATT_MM_BIG", "float32r")]     # Gram + O1
MM_SMALL = _DT_MAP[os.environ.get("ATT_MM_SMALL", "float32r")]  # 256^3 matmuls

# weight buffer layout (per-partition f32 element offsets)
WQ_OFF = 0
WK_OFF = WQ_OFF + H * TC * C     # 2048
WV_OFF = WK_OFF + H * TC * C     # 4096
WO_OFF = WV_OFF + H * TC * C     # 6144
WBUF_W = WO_OFF + TC * C         # 6656


def host_pack_weights(Wq, Wk, Wv, Wout):
    """Pack all weights into the exact [128, WBUF_W] SBUF image."""
    def swz(a):  # [X, 2, 128, Y] -> [128, X*2*Y]
        return np.ascontiguousarray(
            a.reshape(-1, TC, P, C).transpose(2, 0, 1, 3).reshape(P, -1)
        )

    wq = swz(Wq.transpose(0, 2, 1))   # [p, h*tc*d] = Wq[h, d, tc*128+p]
    wk = swz(Wk.transpose(0, 2, 1))
    wv = swz(Wv)                      # [p, h*tc*c] = Wv[h, tc*128+p, c]
    wo = swz(Wout.T[None])            # [p, tc*d] = Wout[d, tc*128+p]
    return np.ascontiguousarray(np.concatenate([wq, wk, wv, wo], axis=1))


def build_bass():
    nc = bacc.Bacc(None, target_bir_lowering=False)

    emb_h = nc.dram_tensor("emb", [N, C], MM_BIG, kind="ExternalInput")
    embT_h = nc.dram_tensor("embT", [C, N], MM_BIG, kind="ExternalInput")
    wbuf_h = nc.dram_tensor("wbuf", [P, WBUF_W], MM_SMALL, kind="ExternalInput")
    o1T_h = nc.dram_tensor("o1T", [C, N], F32, kind="ExternalOutput")
    wts_h = nc.dram_tensor("wts", [C, C], F32, kind="ExternalOutput")

    with tile.TileContext(nc) as tc:
        with (
            tc.tile_pool(name="singles", bufs=1) as singles,
            tc.tile_pool(name="perhead", bufs=2) as perhead,
            tc.tile_pool(name="outs", bufs=3) as outs,
            tc.tile_pool(name="psc", bufs=4, space="PSUM") as psc,
            tc.tile_pool(name="ps", bufs=2, space="PSUM") as ps,
            tc.tile_pool(name="acc", bufs=2, space="PSUM") as acc,
        ):
            # ---- resident SBUF tensors -------------------------------------
            emb_sb = singles.tile([P, KT, C], MM_BIG)    # emb[p*32+t, c]
            embT_sb = singles.tile([P, TC, N], MM_BIG)   # emb[n, t*128+p]
            wbuf_sb = singles.tile([P, WBUF_W], MM_SMALL)
            G_sb = singles.tile([P, TC, C], MM_SMALL)    # G/sqrt(C), [c', (tc,c)]
            S_sb = singles.tile([P, TC, C], MM_SMALL)    # S'/H
            Z_sb = singles.tile([P, TC, C], MM_BIG)
            probs_sb = singles.tile([P, 2 * H, C], F32)  # [i, (2h+mi), j]
            wacc_sb = singles.tile([P, TC, C], F32)
            stat_sb = singles.tile([P, 2 * H], F32)      # sums | sumsqs (h-major)
            rmax_sb = singles.tile([P, H, TC], F32)
            nb_sb = singles.tile([P, H, TC], F32)
            se_sb = singles.tile([P, 2 * H], F32)        # exp row sums
            rse_sb = singles.tile([P, 2 * H], F32)
            scal_sb = singles.tile([P, 8 * H], F32)      # per-pair mu|Esq|var|r
            ones_sb = singles.tile([P, P], F32)
            ident_sb = singles.tile([P, P], F32)
            ident_r_sb = (
                singles.tile([P, P], MM_SMALL, name="ident_r")
                if MM_SMALL != F32
                else None
            )
            eps_sb = singles.tile([P, 1], F32)

            nc.vector.memset(ones_sb[:], 1.0)
            nc.vector.memset(eps_sb[:], EPS)
            make_identity(nc, ident_sb[:])
            if ident_r_sb is not None:
                nc.vector.tensor_copy(out=ident_r_sb[:], in_=ident_sb[:])

            def wq_ap(h, t):
                return wbuf_sb[:, WQ_OFF + (h * TC + t) * C : WQ_OFF + (h * TC + t + 1) * C]

            def wk_ap(h, t):
                return wbuf_sb[:, WK_OFF + (h * TC + t) * C : WK_OFF + (h * TC + t + 1) * C]

            def wv_ap(h, t):
                return wbuf_sb[:, WV_OFF + (h * TC + t) * C : WV_OFF + (h * TC + t + 1) * C]

            def wo_ap(t):
                return wbuf_sb[:, WO_OFF + t * C : WO_OFF + (t + 1) * C]

            # ---- input DMAs (emb chunked so Gram starts early; embT later) -
            # the HWDGE queue drains strictly FIFO, so emission order below is
            # wire order: emb chunks feed the Gram, wk/wq arrive in time for
            # the first head, the rest follows, embT (O1-only) goes last.
            emb_dram = emb_h[:].rearrange("(p t) c -> p t c", p=P)
            TPC = KT // EMB_CHUNKS

            def emb_chunk_dma(ch):
                return nc.sync.dma_start(
                    out=emb_sb[:, ch * TPC : (ch + 1) * TPC, :],
                    in_=emb_dram[:, ch * TPC : (ch + 1) * TPC, :],
                )

            emb_chunk_dma(0)
            emb_chunk_dma(1)
            nc.sync.dma_start(
                out=wbuf_sb[:, WK_OFF:WV_OFF], in_=wbuf_h[:][:, WK_OFF:WV_OFF]
            )
            nc.sync.dma_start(
                out=wbuf_sb[:, WQ_OFF:WK_OFF], in_=wbuf_h[:][:, WQ_OFF:WK_OFF]
            )
            for ch in range(2, EMB_CHUNKS):
                emb_chunk_dma(ch)
            nc.sync.dma_start(
                out=wbuf_sb[:, WV_OFF:WBUF_W], in_=wbuf_h[:][:, WV_OFF:WBUF_W]
            )

            # ---- Gram: G = emb.T @ emb, scaled by 1/sqrt(C) ----------------
            # token-partition per tile t is {p*32+t}; any partition of the
            # 4096 tokens is valid for the Gram contraction.  G is symmetric:
            # compute the upper 128 rows + lower-right block, transpose-copy
            # the rest.
            g0 = ps.tile([P, C], F32, tag="ps", name="g0")
            g1 = ps.tile([P, P], F32, tag="ps", name="g1")
            last_gram = None
            for k in range(KT):
                nc.tensor.matmul(
                    g0[:],
                    lhsT=emb_sb[:, k, 0:P],
                    rhs=emb_sb[:, k, :],
                    start=(k == 0),
                    stop=(k == KT - 1),
                )
                last_gram = nc.tensor.matmul(
                    g1[:],
                    lhsT=emb_sb[:, k, P:C],
                    rhs=emb_sb[:, k, P:C],
                    start=(k == 0),
                    stop=(k == KT - 1),
                )
            nc.vector.tensor_scalar_mul(G_sb[:, 0, :], g0[:], 1.0 / 16.0)
            nc.vector.tensor_scalar_mul(G_sb[:, 1, P:C], g1[:], 1.0 / 16.0)
            gt_ps = ps.tile([P, P], MM_SMALL, tag="ps", name="gt")
            ident_g = ident_r_sb[:] if ident_r_sb is not None else ident_sb[:]
            nc.tensor.transpose(gt_ps[:], G_sb[:, 0, P:C], ident_g)
            nc.vector.tensor_copy(out=G_sb[:, 1, 0:P], in_=gt_ps[:])

            # embT only needed by the O1 epilogue; keep it off the wire until
            # the Gram has consumed emb (DMA bandwidth contention otherwise)
            embt_dma = nc.sync.dma_start(
                out=embT_sb[:], in_=embT_h[:].rearrange("(t p) n -> p t n", p=P)
            )
            _add_dep_helper(
                embt_dma.ins, last_gram.ins, sync=True,
                reason="embT waits for gram to finish reading emb",
            )

            # S' accumulator lives across the whole head loop
            s_acc = [
                acc.tile([P, C], F32, tag="acc", name=f"sacc{i}") for i in range(TC)
            ]

            inv_cc = 1.0 / float(C * C)
            sc_ps = [None] * H

            def emit_head_scores(h):
                U_sb = perhead.tile([P, TC, C], MM_SMALL, tag="u", name=f"u{h}")
                for mc in range(TC):
                    u_ps = ps.tile([P, C], F32, tag="ps")
                    for kc in range(TC):
                        nc.tensor.matmul(
                            u_ps[:],
                            lhsT=G_sb[:, kc, mc * P : (mc + 1) * P],
                            rhs=wk_ap(h, kc),
                            start=(kc == 0),
                            stop=(kc == TC - 1),
                        )
                    nc.vector.tensor_copy(out=U_sb[:, mc, :], in_=u_ps[:])

                p_ = psc.tile([P, TC, C], F32, tag="sc", name=f"sc{h}")
                for mi in range(TC):
                    for kc in range(TC):
                        nc.tensor.matmul(
                            p_[:, mi, :],
                            lhsT=wq_ap(h, kc)[:, mi * P : (mi + 1) * P],
                            rhs=U_sb[:, kc, :],
                            start=(kc == 0),
                            stop=(kc == TC - 1),
                        )
                sc_ps[h] = p_

                # stats stream in behind the score matmuls
                sq_scratch = perhead.tile([P, TC, C], F32, tag="sqs")
                nc.vector.reduce_sum(
                    out=stat_sb[:, h : h + 1],
                    in_=p_[:],
                    axis=mybir.AxisListType.XY,
                )
                nc.scalar.activation(
                    out=sq_scratch[:],
                    in_=p_[:],
                    func=mybir.ActivationFunctionType.Square,
                    accum_out=stat_sb[:, H + h : H + h + 1],
                )
                nc.vector.reduce_max(
                    out=rmax_sb[:, h, :],
                    in_=p_[:],
                    axis=mybir.AxisListType.X,
                )

            def emit_colsum(pair):
                # cross-partition totals for two heads in one tiny matmul:
                # rhs = stat columns {h0, h1, H+h0, H+h1} -> [sum0,sum1,sq0,sq1]
                rhs = perhead.tile([P, 4], F32, tag="csin", name=f"csin{pair[0]}")
                nc.vector.tensor_copy(
                    out=rhs[:].rearrange("p (a b) -> p a b", a=2),
                    in_=stat_sb[:].rearrange("p (a b) -> p a b", a=2)[
                        :, :, pair[0] : pair[0] + 2
                    ],
                )
                cs = ps.tile([P, 4], F32, tag="ps", name=f"cs{pair[0]}")
                nc.tensor.matmul(
                    cs[:], lhsT=ones_sb[:], rhs=rhs[:], start=True, stop=True
                )
                return cs

            def emit_chain(pair, cs):
                # scal layout per pair: [mu(2) | esq(2) | var(2) | r(2)]
                o = pair[0] * 4
                mu = scal_sb[:, o + 0 : o + 2]
                esq = scal_sb[:, o + 2 : o + 4]
                var = scal_sb[:, o + 4 : o + 6]
                rr = scal_sb[:, o + 6 : o + 8]
                nc.vector.tensor_scalar_mul(mu, cs[:, 0:2], inv_cc)
                nc.vector.tensor_scalar_mul(esq, cs[:, 2:4], inv_cc)
                nc.vector.tensor_mul(out=var, in0=mu, in1=mu)
                nc.vector.tensor_tensor(
                    out=var, in0=esq, in1=var, op=mybir.AluOpType.subtract
                )
                nc.scalar.activation(
                    out=var, in_=var,
                    func=mybir.ActivationFunctionType.Sqrt,
                    bias=eps_sb[:],
                )
                nc.vector.reciprocal(out=rr, in_=var)
                # nb[h, mi] = -r_h * rowmax[h, mi] for the two heads
                nc.vector.tensor_tensor(
                    out=nb_sb[:, pair[0] : pair[0] + 2, :],
                    in0=rmax_sb[:, pair[0] : pair[0] + 2, :],
                    in1=rr[:, :, None].to_broadcast([P, 2, TC]),
                    op=mybir.AluOpType.mult,
                )
                nc.vector.tensor_scalar_mul(
                    nb_sb[:, pair[0] : pair[0] + 2, :],
                    nb_sb[:, pair[0] : pair[0] + 2, :],
                    -1.0,
                )

            def rr_ap(h):
                o = (h // 2) * 8 + 6 + (h % 2)
                return scal_sb[:, o : o + 1]

            def emit_phase2(h):
                for mi in range(TC):
                    nc.scalar.activation(
                        out=probs_sb[:, TC * h + mi, :],
                        in_=sc_ps[h][:, mi, :],
                        func=mybir.ActivationFunctionType.Exp,
                        bias=nb_sb[:, h, mi : mi + 1],
                        scale=rr_ap(h),
                        accum_out=se_sb[:, TC * h + mi : TC * h + mi + 1],
                    )
                nc.vector.reciprocal(
                    out=rse_sb[:, TC * h : TC * h + TC],
                    in_=se_sb[:, TC * h : TC * h + TC],
                )
                nc.vector.tensor_tensor(
                    out=probs_sb[:, TC * h : TC * h + TC, :],
                    in0=probs_sb[:, TC * h : TC * h + TC, :],
                    in1=rse_sb[:, TC * h : TC * h + TC, None].to_broadcast(
                        [P, TC, C]
                    ),
                    op=mybir.AluOpType.mult,
                )
                probsT_sb = perhead.tile(
                    [P, TC, C], MM_SMALL, tag="probsT", name=f"pt{h}"
                )
                for ti in range(TC):
                    for tj in range(TC):
                        t_ps = ps.tile([P, P], F32, tag="ps")
                        nc.tensor.transpose(
                            t_ps[:],
                            probs_sb[:, TC * h + ti, tj * P : (tj + 1) * P],
                            ident_sb[:],
                        )
                        nc.vector.tensor_copy(
                            out=probsT_sb[:, tj, ti * P : (ti + 1) * P], in_=t_ps[:]
                        )
                for mi in range(TC):
                    for kj in range(TC):
                        nc.tensor.matmul(
                            s_acc[mi][:],
                            lhsT=probsT_sb[:, kj, mi * P : (mi + 1) * P],
                            rhs=wv_ap(h, kj),
                            start=(h == 0 and kj == 0),
                            stop=(h == H - 1 and kj == TC - 1),
                        )

            # interleave so PE never waits on a norm chain:
            emit_head_scores(0)
            emit_head_scores(1)
            emit_head_scores(2)
            cs01 = emit_colsum((0, 1))
            emit_chain((0, 1), cs01)
            emit_head_scores(3)
            emit_phase2(0)
            emit_phase2(1)
            cs23 = emit_colsum((2, 3))
            emit_chain((2, 3), cs23)
            emit_phase2(2)
            emit_phase2(3)

            # weights output: mean over heads via a strided free-dim reduce
            nc.vector.reduce_sum(
                out=wacc_sb[:],
                in_=probs_sb[:].rearrange("p (h m) j -> p m j h", h=H),
                axis=mybir.AxisListType.X,
            )
            nc.gpsimd.tensor_scalar_mul(wacc_sb[:], wacc_sb[:], 1.0 / H)
            nc.sync.dma_start(
                out=wts_h[:].rearrange("(t p) j -> p t j", p=P), in_=wacc_sb[:]
            )

            # ---- epilogue: Z then O1 ---------------------------------------
            for mi in range(TC):
                nc.vector.tensor_scalar_mul(S_sb[:, mi, :], s_acc[mi][:], 1.0 / H)
            for mc in range(TC):
                z_ps = ps.tile([P, C], F32, tag="ps")
                for ki in range(TC):
                    nc.tensor.matmul(
                        z_ps[:],
                        lhsT=S_sb[:, ki, mc * P : (mc + 1) * P],
                        rhs=wo_ap(ki),
                        start=(ki == 0),
                        stop=(ki == TC - 1),
                    )
                nc.vector.tensor_copy(out=Z_sb[:, mc, :], in_=z_ps[:])

            # O1.T[d, n] = sum_c Z[c, d] * embT[c, n]
            for md in range(TC):
                for nch in range(NCH):
                    o_ps = ps.tile([P, 512], F32, tag="ps")
                    for kc in range(TC):
                        nc.tensor.matmul(
                            o_ps[:],
                            lhsT=Z_sb[:, kc, md * P : (md + 1) * P],
                            rhs=embT_sb[:, kc, nch * 512 : (nch + 1) * 512],
                            start=(kc == 0),
                            stop=(kc == TC - 1),
                        )
                    o_sb = outs.tile([P, 512], F32, tag="o1")
                    nc.vector.tensor_copy(out=o_sb[:], in_=o_ps[:])
                    nc.sync.dma_start(
                        out=o1T_h[:][
                            md * P : (md + 1) * P, nch * 512 : (nch + 1) * 512
                        ],
                        in_=o_sb[:],
                    )

    nc.compile()
    return nc


_NC_CACHE = None


def host_in_maps(emb1, Wq, Wk, Wv, Wout):
    wbuf = host_pack_weights(Wq, Wk, Wv, Wout)
    in_maps = []
    for b in range(B):
        in_maps.append(
            {
                "emb": np.ascontiguousarray(emb1[b]),
                "embT": np.ascontiguousarray(emb1[b].T),
                "wbuf": wbuf,
            }
        )
    return in_maps


def kernel(emb1, Wq, Wk, Wv, Wout):
    global _NC_CACHE
    emb1 = np.ascontiguousarray(np.asarray(emb1, dtype=np.float32))
    Wq = np.asarray(Wq, dtype=np.float32)
    Wk = np.asarray(Wk, dtype=np.float32)
    Wv = np.asarray(Wv, dtype=np.float32)
    Wout = np.asarray(Wout, dtype=np.float32)

    if _NC_CACHE is None:
        _NC_CACHE = build_bass()
    nc = _NC_CACHE

    in_maps = host_in_maps(emb1, Wq, Wk, Wv, Wout)
    res = run_bass_kernel_spmd(nc, in_maps, core_ids=list(range(B)))

    O1 = np.empty((B, N, C), dtype=np.float32)
    weights = np.empty((B, C, C), dtype=np.float32)
    for b in range(B):
        O1[b] = res.results[b]["o1T"].T
        weights[b] = res.results[b]["wts"]
    return O1, weights


# revision 22
# speedup vs baseline: 1.0535x; 1.0535x over previous
"""Trainium2 Bass kernel for the per-head channel-attention module.

Math (per batch b, all fp32):
  Q = emb @ Wq[h].T, K = emb @ Wk[h].T        [N, C] each
  scores_h = Q.T @ K / sqrt(C)                [C, C]
  probs = softmax(InstanceNorm(scores), -1)
  weights = mean_h probs                      [C, C]   (output 2)
  O1 = (1/H sum_h probs_h @ V_h-chain) @ Wout [N, C]   (output 1)

Key restructure: scores contract over tokens N, so
  scores_h = Wq_h @ G' @ Wk_h.T  with  G' = (emb.T @ emb) / sqrt(C)
and the output path collapses to
  O1 = emb @ Z;  S'[i,c] = sum_h (probs_h @ Wv_h)[i,c];
  Z[c,d] = (1/H) sum_i S'[i,c] * Wout[d,i]
~11.3 GFLOP/batch -> ~1.5 GFLOP/batch.  InstanceNorm's mean subtraction
cancels inside the row softmax, so only r = rsqrt(var+eps) is needed.
G is exactly symmetric, so its lower-left block is a transpose copy.

Sharding: data-parallel, one batch per NeuronCore (B=8, 8 cores).
Host pre-transposes emb (embT) and pre-swizzles the weight matrices into
one SBUF-layout buffer so every DMA is a few large contiguous runs.
Per-core outputs: o1T = O1[b].T (host transposes back) and wts = weights[b].
"""

import os

import numpy as np

import concourse.bacc as bacc
import concourse.bass as bass
import concourse.mybir as mybir
import concourse.tile as tile
from concourse.bass import _add_dep_helper
from concourse.bass_utils import run_bass_kernel_spmd
from concourse.masks import make_identity

B, N, C, H = 8, 4096, 256, 4
EPS = 1e-5
P = 128
TC = C // P          # 2 c-tiles
KT = N // P          # 32 token-tiles
NCH = N // 512       # 8 chunks of 512 tokens for the final matmul
EMB_CHUNKS = 4
F32 = mybir.dt.float32

# Matmul dtype knobs. float32 = exact (4 cyc/row); float32r = fast (~2 cyc/row
# measured) with relaxed multiply precision (~2e-4 rel err end to end).
_DT_MAP = {"float32": mybir.dt.float32, "float32r": mybir.dt.float32r}
MM_BIG = _DT_MAP[os.environ.get("ATT_MM_BIG", "float32r")]     # Gram + O1
MM_SMALL = _DT_MAP[os.environ.get("ATT_MM_SMALL", "float32r")]  # 256^3 matmuls

# weight buffer layout (per-partition f32 element offsets)
WQ_OFF = 0
WK_OFF = WQ_OFF + H * TC * C     # 2048
WV_OFF = WK_OFF + H * TC * C     # 4096
WO_OFF = WV_OFF + H * TC * C     # 6144
WBUF_W = WO_OFF + TC * C         # 6656


def host_pack_weights(Wq, Wk, Wv, Wout):
    """Pack all weights into the exact [128, WBUF_W] SBUF image."""
    def swz(a):  # [X, 2, 128, Y] -> [128, X*2*Y]
        return np.ascontiguousarray(
            a.reshape(-1, TC, P, C).transpose(2, 0, 1, 3).reshape(P, -1)
        )

    wq = swz(Wq.transpose(0, 2, 1))   # [p, h*tc*d] = Wq[h, d, tc*128+p]
    wk = swz(Wk.transpose(0, 2, 1))
    wv = swz(Wv)                      # [p, h*tc*c] = Wv[h, tc*128+p, c]
    wo = swz(Wout.T[None])            # [p, tc*d] = Wout[d, tc*128+p]
    return np.ascontiguousarray(np.concatenate([wq, wk, wv, wo], axis=1))


def build_bass():
    nc = bacc.Bacc(None, target_bir_lowering=False)

    emb_h = nc.dram_tensor("emb", [N, C], MM_BIG, kind="ExternalInput")
    embT_h = nc.dram_tensor("embT", [C, N], MM_BIG, kind="ExternalInput")
    wbuf_h = nc.dram_tensor("wbuf", [P, WBUF_W], MM_SMALL, kind="ExternalInput")
    o1T_h = nc.dram_tensor("o1T", [C, N], F32, kind="ExternalOutput")
    wts_h = nc.dram_tensor("wts", [C, C], F32, kind="ExternalOutput")

    with tile.TileContext(nc) as tc:
        with (
            tc.tile_pool(name="singles", bufs=1) as singles,
            tc.tile_pool(name="perhead", bufs=2) as perhead,
            tc.tile_pool(name="outs", bufs=3) as outs,
            tc.tile_pool(name="psc", bufs=4, space="PSUM") as psc,
            tc.tile_pool(name="ps", bufs=2, space="PSUM") as ps,
            tc.tile_pool(name="acc", bufs=2, space="PSUM") as acc,
        ):
            # ---- resident SBUF tensors -------------------------------------
            emb_sb = singles.tile([P, KT, C], MM_BIG)    # emb[p*32+t, c]
            embT_sb = singles.tile([P, TC, N], MM_BIG)   # emb[n, t*128+p]
            wbuf_sb = singles.tile([P, WBUF_W], MM_SMALL)
            G_sb = singles.tile([P, TC, C], MM_SMALL)    # G/sqrt(C), [c', (tc,c)]
            S_sb = singles.tile([P, TC, C], MM_SMALL)    # S'/H
            Z_sb = singles.tile([P, TC, C], MM_BIG)
            probs_sb = singles.tile([P, 2 * H, C], F32)  # [i, (2h+mi), j]
            wacc_sb = singles.tile([P, TC, C], F32)
            stat_sb = singles.tile([P, H, 3], F32)       # mean|var|mean^2 per head
            bnst_sb = singles.tile([P, H, TC, 6], F32)   # bn_stats scratch
            se_sb = singles.tile([P, 2 * H], F32)        # exp row sums
            rse_sb = singles.tile([P, 2 * H], F32)
            scal_sb = singles.tile([P, 8 * H], F32)      # per-pair mu|Esq|var|r
            ones_sb = singles.tile([P, P], F32)
            ident_sb = singles.tile([P, P], F32)
            ident_r_sb = (
                singles.tile([P, P], MM_SMALL, name="ident_r")
                if MM_SMALL != F32
                else None
            )
            eps_sb = singles.tile([P, 1], F32)

            nc.vector.memset(ones_sb[:], 1.0)
            nc.vector.memset(eps_sb[:], EPS)
            make_identity(nc, ident_sb[:])
            if ident_r_sb is not None:
                nc.vector.tensor_copy(out=ident_r_sb[:], in_=ident_sb[:])

            def wq_ap(h, t):
                return wbuf_sb[:, WQ_OFF + (h * TC + t) * C : WQ_OFF + (h * TC + t + 1) * C]

            def wk_ap(h, t):
                return wbuf_sb[:, WK_OFF + (h * TC + t) * C : WK_OFF + (h * TC + t + 1) * C]

            def wv_ap(h, t):
                return wbuf_sb[:, WV_OFF + (h * TC + t) * C : WV_OFF + (h * TC + t + 1) * C]

            def wo_ap(t):
                return wbuf_sb[:, WO_OFF + t * C : WO_OFF + (t + 1) * C]

            # ---- input DMAs (emb chunked so Gram starts early; embT later) -
            # the HWDGE queue drains strictly FIFO, so emission order below is
            # wire order: emb chunks feed the Gram, wk/wq arrive in time for
            # the first head, the rest follows, embT (O1-only) goes last.
            emb_dram = emb_h[:].rearrange("(p t) c -> p t c", p=P)
            bounds = [0, 4, 12, 22, KT]
            for ci in range(len(bounds) - 1):
                nc.sync.dma_start(
                    out=emb_sb[:, bounds[ci] : bounds[ci + 1], :],
                    in_=emb_dram[:, bounds[ci] : bounds[ci + 1], :],
                )
            nc.sync.dma_start(
                out=wbuf_sb[:, WK_OFF:WV_OFF], in_=wbuf_h[:][:, WK_OFF:WV_OFF]
            )
            nc.sync.dma_start(
                out=wbuf_sb[:, WQ_OFF:WK_OFF], in_=wbuf_h[:][:, WQ_OFF:WK_OFF]
            )
            nc.sync.dma_start(
                out=wbuf_sb[:, WV_OFF:WBUF_W], in_=wbuf_h[:][:, WV_OFF:WBUF_W]
            )

            # ---- Gram: G = emb.T @ emb, scaled by 1/sqrt(C) ----------------
            # token-partition per tile t is {p*32+t}; any partition of the
            # 4096 tokens is valid for the Gram contraction.  G is symmetric:
            # compute the upper 128 rows + lower-right block, transpose-copy
            # the rest.
            g0 = ps.tile([P, C], F32, tag="ps", name="g0")
            g1 = ps.tile([P, P], F32, tag="ps", name="g1")
            last_gram = None
            for k in range(KT):
                nc.tensor.matmul(
                    g0[:],
                    lhsT=emb_sb[:, k, 0:P],
                    rhs=emb_sb[:, k, :],
                    start=(k == 0),
                    stop=(k == KT - 1),
                )
                last_gram = nc.tensor.matmul(
                    g1[:],
                    lhsT=emb_sb[:, k, P:C],
                    rhs=emb_sb[:, k, P:C],
                    start=(k == 0),
                    stop=(k == KT - 1),
                )
            nc.vector.tensor_scalar_mul(G_sb[:, 0, :], g0[:], 1.0 / 16.0)
            nc.vector.tensor_scalar_mul(G_sb[:, 1, P:C], g1[:], 1.0 / 16.0)
            gt_ps = ps.tile([P, P], MM_SMALL, tag="ps", name="gt")
            ident_g = ident_r_sb[:] if ident_r_sb is not None else ident_sb[:]
            nc.tensor.transpose(gt_ps[:], G_sb[:, 0, P:C], ident_g)
            nc.vector.tensor_copy(out=G_sb[:, 1, 0:P], in_=gt_ps[:])

            # embT rides last on the FIFO DMA queue; it lands in the quiet
            # window before the O1 epilogue needs it
            nc.sync.dma_start(
                out=embT_sb[:], in_=embT_h[:].rearrange("(t p) n -> p t n", p=P)
            )

            # S' accumulator lives across the whole head loop
            s_acc = [
                acc.tile([P, C], F32, tag="acc", name=f"sacc{i}") for i in range(TC)
            ]

            inv_cc = 1.0 / float(C * C)
            sc_ps = [None] * H

            def emit_head_scores(h):
                U_sb = perhead.tile([P, TC, C], MM_SMALL, tag="u", name=f"u{h}")
                for mc in range(TC):
                    u_ps = ps.tile([P, C], F32, tag="ps")
                    for kc in range(TC):
                        nc.tensor.matmul(
                            u_ps[:],
                            lhsT=G_sb[:, kc, mc * P : (mc + 1) * P],
                            rhs=wk_ap(h, kc),
                            start=(kc == 0),
                            stop=(kc == TC - 1),
                        )
                    nc.vector.tensor_copy(out=U_sb[:, mc, :], in_=u_ps[:])

                p_ = psc.tile([P, TC, C], F32, tag="sc", name=f"sc{h}")
                for mi in range(TC):
                    for kc in range(TC):
                        nc.tensor.matmul(
                            p_[:, mi, :],
                            lhsT=wq_ap(h, kc)[:, mi * P : (mi + 1) * P],
                            rhs=U_sb[:, kc, :],
                            start=(kc == 0),
                            stop=(kc == TC - 1),
                        )
                sc_ps[h] = p_

                # per-partition mean/var in two DVE passes (no ACT tables)
                for mi in range(TC):
                    nc.vector.bn_stats(
                        out=bnst_sb[:, h, mi, :], in_=p_[:, mi, :]
                    )
                nc.vector.bn_aggr(out=stat_sb[:, h, 0:2], in_=bnst_sb[:, h, :, :])
                nc.vector.tensor_mul(
                    out=stat_sb[:, h, 2:3],
                    in0=stat_sb[:, h, 0:1],
                    in1=stat_sb[:, h, 0:1],
                )

            def emit_colsum(pair):
                # cross-partition sums of [mean, var, mean^2] for two heads
                cs = ps.tile([P, 2, 3], F32, tag="ps", name=f"cs{pair[0]}")
                nc.tensor.matmul(
                    cs[:],
                    lhsT=ones_sb[:],
                    rhs=stat_sb[:, pair[0] : pair[0] + 2, :],
                    start=True,
                    stop=True,
                )
                return cs

            def emit_chain(pair, cs):
                # combined var over the CxC map from per-partition stats:
                # var = E_p[var_p] + E_p[mean_p^2] - (E_p[mean_p])^2
                # scal layout per pair: [mu(2) | vtmp(2) | var(2) | r(2)]
                o = pair[0] * 4
                mu = scal_sb[:, o + 0 : o + 2]
                vt = scal_sb[:, o + 2 : o + 4]
                var = scal_sb[:, o + 4 : o + 6]
                rr = scal_sb[:, o + 6 : o + 8]
                cssb = perhead.tile([P, 2, 3], F32, tag="cssb", name=f"cssb{pair[0]}")
                nc.vector.tensor_copy(out=cssb[:], in_=cs[:])
                nc.vector.tensor_scalar_mul(mu, cssb[:, :, 0], 1.0 / P)
                nc.vector.tensor_tensor(
                    out=vt, in0=cssb[:, :, 1], in1=cssb[:, :, 2],
                    op=mybir.AluOpType.add,
                )
                nc.vector.tensor_scalar_mul(vt, vt, 1.0 / P)
                nc.vector.tensor_mul(out=var, in0=mu, in1=mu)
                nc.vector.tensor_tensor(
                    out=var, in0=vt, in1=var, op=mybir.AluOpType.subtract
                )
                nc.scalar.activation(
                    out=var, in_=var,
                    func=mybir.ActivationFunctionType.Sqrt,
                    bias=eps_sb[:],
                )
                nc.vector.reciprocal(out=rr, in_=var)

            def rr_ap(h):
                o = (h // 2) * 8 + 6 + (h % 2)
                return scal_sb[:, o : o + 1]

            def emit_phase2(h):
                # scores*r is ~N(0,1) over the map: exp never overflows, so
                # skip the usual rowmax subtraction (it cancels in softmax)
                for mi in range(TC):
                    nc.scalar.activation(
                        out=probs_sb[:, TC * h + mi, :],
                        in_=sc_ps[h][:, mi, :],
                        func=mybir.ActivationFunctionType.Exp,
                        scale=rr_ap(h),
                        accum_out=se_sb[:, TC * h + mi : TC * h + mi + 1],
                    )
                nc.vector.reciprocal(
                    out=rse_sb[:, TC * h : TC * h + TC],
                    in_=se_sb[:, TC * h : TC * h + TC],
                )
                nc.vector.tensor_tensor(
                    out=probs_sb[:, TC * h : TC * h + TC, :],
                    in0=probs_sb[:, TC * h : TC * h + TC, :],
                    in1=rse_sb[:, TC * h : TC * h + TC, None].to_broadcast(
                        [P, TC, C]
                    ),
                    op=mybir.AluOpType.mult,
                )
                probsT_sb = perhead.tile(
                    [P, TC, C], MM_SMALL, tag="probsT", name=f"pt{h}"
                )
                for ti in range(TC):
                    for tj in range(TC):
                        t_ps = ps.tile([P, P], F32, tag="ps")
                        nc.tensor.transpose(
                            t_ps[:],
                            probs_sb[:, TC * h + ti, tj * P : (tj + 1) * P],
                            ident_sb[:],
                        )
                        nc.vector.tensor_copy(
                            out=probsT_sb[:, tj, ti * P : (ti + 1) * P], in_=t_ps[:]
                        )
                for mi in range(TC):
                    for kj in range(TC):
                        nc.tensor.matmul(
                            s_acc[mi][:],
                            lhsT=probsT_sb[:, kj, mi * P : (mi + 1) * P],
                            rhs=wv_ap(h, kj),
                            start=(h == 0 and kj == 0),
                            stop=(h == H - 1 and kj == TC - 1),
                        )

            # interleave so PE never waits on a norm chain:
            emit_head_scores(0)
            emit_head_scores(1)
            emit_head_scores(2)
            cs01 = emit_colsum((0, 1))
            emit_chain((0, 1), cs01)
            emit_head_scores(3)
            emit_phase2(0)
            emit_phase2(1)
            cs23 = emit_colsum((2, 3))
            emit_chain((2, 3), cs23)
            emit_phase2(2)
            emit_phase2(3)

            # ---- epilogue: Z then O1 ---------------------------------------
            for mi in range(TC):
                nc.vector.tensor_scalar_mul(S_sb[:, mi, :], s_acc[mi][:], 1.0 / H)
            for mc in range(TC):
                z_ps = ps.tile([P, C], F32, tag="ps")
                for ki in range(TC):
                    nc.tensor.matmul(
                        z_ps[:],
                        lhsT=S_sb[:, ki, mc * P : (mc + 1) * P],
                        rhs=wo_ap(ki),
                        start=(ki == 0),
                        stop=(ki == TC - 1),
                    )
                nc.vector.tensor_copy(out=Z_sb[:, mc, :], in_=z_ps[:])

            # O1.T[d, n] = sum_c Z[c, d] * embT[c, n]
            for md in range(TC):
                for nch in range(NCH):
                    o_ps = ps.tile([P, 512], F32, tag="ps")
                    for kc in range(TC):
                        nc.tensor.matmul(
                            o_ps[:],
                            lhsT=Z_sb[:, kc, md * P : (md + 1) * P],
                            rhs=embT_sb[:, kc, nch * 512 : (nch + 1) * 512],
                            start=(kc == 0),
                            stop=(kc == TC - 1),
                        )
                    o_sb = outs.tile([P, 512], F32, tag="o1")
                    nc.vector.tensor_copy(out=o_sb[:], in_=o_ps[:])
                    nc.sync.dma_start(
                        out=o1T_h[:][
                            md * P : (md + 1) * P, nch * 512 : (nch + 1) * 512
                        ],
                        in_=o_sb[:],
                    )

            # weights output last: off every critical path
            nc.vector.reduce_sum(
                out=wacc_sb[:],
                in_=probs_sb[:].rearrange("p (h m) j -> p m j h", h=H),
                axis=mybir.AxisListType.X,
            )
            nc.gpsimd.tensor_scalar_mul(wacc_sb[:], wacc_sb[:], 1.0 / H)
            nc.sync.dma_start(
                out=wts_h[:].rearrange("(t p) j -> p t j", p=P), in_=wacc_sb[:]
            )

    nc.compile()
    return nc


_NC_CACHE = None


def host_in_maps(emb1, Wq, Wk, Wv, Wout):
    wbuf = host_pack_weights(Wq, Wk, Wv, Wout)
    in_maps = []
    for b in range(B):
        in_maps.append(
            {
                "emb": np.ascontiguousarray(emb1[b]),
                "embT": np.ascontiguousarray(emb1[b].T),
                "wbuf": wbuf,
            }
        )
    return in_maps


def kernel(emb1, Wq, Wk, Wv, Wout):
    global _NC_CACHE
    emb1 = np.ascontiguousarray(np.asarray(emb1, dtype=np.float32))
    Wq = np.asarray(Wq, dtype=np.float32)
    Wk = np.asarray(Wk, dtype=np.float32)
    Wv = np.asarray(Wv, dtype=np.float32)
    Wout = np.asarray(Wout, dtype=np.float32)

    if _NC_CACHE is None:
        _NC_CACHE = build_bass()
    nc = _NC_CACHE

    in_maps = host_in_maps(emb1, Wq, Wk, Wv, Wout)
    res = run_bass_kernel_spmd(nc, in_maps, core_ids=list(range(B)))

    O1 = np.empty((B, N, C), dtype=np.float32)
    weights = np.empty((B, C, C), dtype=np.float32)
    for b in range(B):
        O1[b] = res.results[b]["o1T"].T
        weights[b] = res.results[b]["wts"]
    return O1, weights
